# revision 12
# baseline (speedup 1.0000x reference)
"""Trainium2 Bass kernel for nn_Decoder (attention pooling over sorted segments + tiny MLPs).

Strategy (data-parallel over segments, 8 cores):
  - Core c owns segments [c*512, (c+1)*512) and the contiguous node range covering them
    (batch_clu is sorted, so each core's nodes are a contiguous slice).
  - Host pre-arranges per-core inputs:
      * x nodes cast to f16, laid out [block, partition(node%128), tile, d] so each DMA
        is contiguous per partition.
      * a windowed one-hot "assignment" matrix per block (window of WB=16 segments
        starting at the block's first segment), mask folded in, f16.
      * a fold matrix F mapping (block, window-col) staging slots -> the core's 512
        segments (applied on-device as a matmul at the end).
  - Device per block (2048 nodes = 16 tiles of 128):
      gate  = reduce_d(x_f16 * w_rep)           (DVE, f16 2x mode, two-stage reduce)
      e     = exp(gate)                         (ACT, f32 -> f16)
      won   = onehot * e                        (DVE)
      S^T  += won_t^T @ x_t   (PE, accumulates [WB x 128] per block into group PSUM)
      den  += won_t^T @ ones  (PE, [WB x 1])
    Groups of 8 blocks share one [128 x 128] PSUM tile (disjoint 16-partition slices),
    copied once per group to SBUF staging (f16).
  - Epilogue: fold staging -> per-segment sums via F matmuls, normalize by
    max(den,1e-12), run both MLP heads fully transposed (no PE transposes needed
    anywhere), scale by mean(dist_embedding), DMA out [6 x 512] + [1 x 512].
  - No max-subtraction in the segment softmax: softmax is shift-invariant and
    gate ~ N(0,1), so exp() is well-conditioned; gate bias cancels in the ratio.
"""

import sys

sys.path.insert(0, "/opt/trn_rl_repo")

import numpy as np
from contextlib import ExitStack

import concourse.bass as bass
import concourse.bacc as bacc
import concourse.mybir as mybir
import concourse.tile as tile
from concourse.bass_utils import run_bass_kernel_spmd

P = 128          # partitions / nodes per tile
D = 128          # feature dim
TPB = 16         # tiles per block
NPB = P * TPB    # nodes per block (2048)
WB = 16          # segment window width per block
BPG = 3          # blocks per PSUM group (PE out base partition must be 0/32/64)
GSTRIDE = 32     # partition stride between blocks within a group
NCORES = 8
B_SEG = 4096
SEGC = B_SEG // NCORES  # segments per core (512)

F32 = mybir.dt.float32
F16 = mybir.dt.float16
AX = mybir.AxisListType
ALU = mybir.AluOpType
AF = mybir.ActivationFunctionType


def build_program(NB: int):
    """Build the single SPMD Bass program (same for all 8 cores)."""
    assert NB % BPG == 0
    NG = NB // BPG

    # Bacc (not raw Bass): its compile() pass splits multi-sem waits into
    # event-semaphore chains — walrus rejects any instruction with >1 wait.
    nc = bacc.Bacc(None)

    xh = nc.declare_dram_parameter("xh", [NB, P, TPB, D], F16, isOutput=False)
    ohh = nc.declare_dram_parameter("ohh", [NB, P, TPB, WB], F16, isOutput=False)
    fh = nc.declare_dram_parameter("fh", [NG, P, SEGC], F16, isOutput=False)
    wrep = nc.declare_dram_parameter("wrep", [P, D], F16, isOutput=False)
    w1h = nc.declare_dram_parameter("w1h", [D, D], F32, isOutput=False)
    b1h = nc.declare_dram_parameter("b1h", [D, 1], F32, isOutput=False)
    w2h = nc.declare_dram_parameter("w2h", [D, 6], F32, isOutput=False)
    b2h = nc.declare_dram_parameter("b2h", [6, 1], F32, isOutput=False)
    w1nh = nc.declare_dram_parameter("w1nh", [D, D], F32, isOutput=False)
    b1nh = nc.declare_dram_parameter("b1nh", [D, 1], F32, isOutput=False)
    w2nh = nc.declare_dram_parameter("w2nh", [D, 1], F32, isOutput=False)
    b2nh = nc.declare_dram_parameter("b2nh", [1, 1], F32, isOutput=False)
    deh = nc.declare_dram_parameter("deh", [P, 48], F32, isOutput=False)
    dnh = nc.declare_dram_parameter("dnh", [P, 8], F32, isOutput=False)
    ovt = nc.declare_dram_parameter("ovt", [6, SEGC], F32, isOutput=True)
    ont = nc.declare_dram_parameter("ont", [1, SEGC], F32, isOutput=True)

    with tile.TileContext(nc) as tc, ExitStack() as ctx:
        cst = ctx.enter_context(tc.tile_pool(name="cst", bufs=1))

        wrep_sb = cst.tile([P, D], F16)
        nc.sync.dma_start(wrep_sb[:], wrep[:])
        w1_sb = cst.tile([D, D], F32)
        nc.sync.dma_start(w1_sb[:], w1h[:])
        b1_sb = cst.tile([D, 1], F32)
        nc.sync.dma_start(b1_sb[:], b1h[:])
        w2_sb = cst.tile([D, 6], F32)
        nc.sync.dma_start(w2_sb[:], w2h[:])
        b2_sb = cst.tile([6, 1], F32)
        nc.sync.dma_start(b2_sb[:], b2h[:])
        w1n_sb = cst.tile([D, D], F32)
        nc.sync.dma_start(w1n_sb[:], w1nh[:])
        b1n_sb = cst.tile([D, 1], F32)
        nc.sync.dma_start(b1n_sb[:], b1nh[:])
        w2n_sb = cst.tile([D, 1], F32)
        nc.sync.dma_start(w2n_sb[:], w2nh[:])
        b2n_sb = cst.tile([1, 1], F32)
        nc.sync.dma_start(b2n_sb[:], b2nh[:])
        de_sb = cst.tile([P, 48], F32)
        nc.sync.dma_start(de_sb[:], deh[:])
        dn_sb = cst.tile([P, 8], F32)
        nc.sync.dma_start(dn_sb[:], dnh[:])

        ones16 = cst.tile([P, 1], F16)
        nc.vector.memset(ones16[:], 1.0)
        ones32 = cst.tile([P, 1], F32)
        nc.vector.memset(ones32[:], 1.0)
        ones_row = cst.tile([1, P], F32)
        nc.vector.memset(ones_row[:], 1.0)

        # staging for per-(block,window-col) partial sums
        stg = ctx.enter_context(tc.tile_pool(name="stg", bufs=1))
        stage_g = [
            stg.tile([P, D], F16, tag=f"st{g}", name=f"stage{g}") for g in range(NG)
        ]
        den_g = [
            stg.tile([P, 1], F16, tag=f"dn{g}", name=f"den{g}") for g in range(NG)
        ]

        with (
            tc.tile_pool(name="lp", bufs=3) as lp,
            tc.tile_pool(name="lps", bufs=2) as lps,
            tc.tile_pool(name="pl", bufs=2, space="PSUM") as pl,
        ):
            for g in range(NG):
                S_ps = pl.tile([P, D], F32, tag="S")
                d_ps = pl.tile([P, 1], F32, tag="d")
                # blocks only cover partitions [32*bb, 32*bb+16); zero the rest
                nc.scalar.memzero(S_ps[:])
                nc.scalar.memzero(d_ps[:])
                for bb in range(BPG):
                    b = g * BPG + bb
                    x_sb = lp.tile([P, TPB, D], F16, tag="x")
                    nc.sync.dma_start(x_sb[:], xh[b])
                    oh_sb = lp.tile([P, TPB, WB], F16, tag="oh")
                    nc.sync.dma_start(oh_sb[:], ohh[b])

                    prod = lps.tile([P, TPB, D], F16, tag="prod")
                    nc.vector.tensor_tensor(
                        out=prod[:],
                        in0=x_sb[:],
                        in1=wrep_sb[:, None, :].to_broadcast([P, TPB, D]),
                        op=ALU.mult,
                    )
                    pg = lps.tile([P, TPB, 8], F16, tag="pg")
                    with nc.allow_low_precision(
                        reason="f16 partial sums of 16 terms; validated ~5e-4"
                    ):
                        nc.vector.tensor_reduce(
                            out=pg[:],
                            in_=prod[:].rearrange("p t (g8 e) -> p t g8 e", e=16),
                            axis=AX.X,
                            op=ALU.add,
                        )
                    gates = lps.tile([P, TPB], F32, tag="gates")
                    nc.vector.tensor_reduce(
                        out=gates[:], in_=pg[:], axis=AX.X, op=ALU.add
                    )
                    e16 = lps.tile([P, TPB], F16, tag="e16")
                    nc.scalar.activation(out=e16[:], in_=gates[:], func=AF.Exp)
                    won = lps.tile([P, TPB, WB], F16, tag="won")
                    nc.vector.tensor_tensor(
                        out=won[:],
                        in0=oh_sb[:],
                        in1=e16[:, :, None].to_broadcast([P, TPB, WB]),
                        op=ALU.mult,
                    )
                    p0 = bb * GSTRIDE
                    for t in range(TPB):
                        nc.tensor.matmul(
                            S_ps[p0 : p0 + WB, :],
                            lhsT=won[:, t, :],
                            rhs=x_sb[:, t, :],
                            start=(t == 0),
                            stop=(t == TPB - 1),
                        )
                        nc.tensor.matmul(
                            d_ps[p0 : p0 + WB, :],
                            lhsT=won[:, t, :],
                            rhs=ones16[:, :],
                            start=(t == 0),
                            stop=(t == TPB - 1),
                        )
                nc.scalar.copy(out=stage_g[g][:], in_=S_ps[:])
                nc.scalar.copy(out=den_g[g][:], in_=d_ps[:])

        # ---- epilogue ----
        with (
            tc.tile_pool(name="ep", bufs=2) as ep,
            tc.tile_pool(name="pA", bufs=1, space="PSUM") as pA,
            tc.tile_pool(name="pC", bufs=2, space="PSUM") as pC,
        ):
            S_acc = pA.tile([P, SEGC], F32, tag="Sacc")
            den_acc = pA.tile([1, SEGC], F32, tag="dacc")
            for g in range(NG):
                f_sb = ep.tile([P, SEGC], F16, tag="f")
                nc.sync.dma_start(f_sb[:], fh[g])
                nc.tensor.matmul(
                    S_acc[:],
                    lhsT=stage_g[g][:],
                    rhs=f_sb[:],
                    start=(g == 0),
                    stop=(g == NG - 1),
                )
                nc.tensor.matmul(
                    den_acc[:],
                    lhsT=den_g[g][:],
                    rhs=f_sb[:],
                    start=(g == 0),
                    stop=(g == NG - 1),
                )

            dmax = ep.tile([1, SEGC], F32, tag="dmax")
            nc.vector.tensor_scalar_max(out=dmax[:], in0=den_acc[:], scalar1=1e-12)
            recip = ep.tile([1, SEGC], F32, tag="recip")
            nc.vector.reciprocal(recip[:], dmax[:])
            bc = pC.tile([P, SEGC], F32, tag="work")
            nc.tensor.matmul(bc[:], lhsT=ones_row[:], rhs=recip[:], start=True, stop=True)
            bc_sb = ep.tile([P, SEGC], F32, tag="bc_sb")
            nc.scalar.copy(out=bc_sb[:], in_=bc[:])
            xsT = ep.tile([P, SEGC], F32, tag="xsT")
            nc.vector.tensor_tensor(out=xsT[:], in0=S_acc[:], in1=bc_sb[:], op=ALU.mult)

            # vel head: hT = relu(w1^T xs^T + b1); v^T = w2^T hT; out = v^T*mean + b2*mean
            h_ps = pC.tile([P, SEGC], F32, tag="work")
            nc.tensor.matmul(h_ps[:], lhsT=w1_sb[:], rhs=xsT[:], start=True, stop=True)
            hT = ep.tile([P, SEGC], F32, tag="hT")
            nc.scalar.activation(out=hT[:], in_=h_ps[:], func=AF.Relu, bias=b1_sb[:, :1])
            v_ps = pC.tile([6, SEGC], F32, tag="sm")
            nc.tensor.matmul(v_ps[:], lhsT=w2_sb[:], rhs=hT[:], start=True, stop=True)

            dpart = ep.tile([P, 6], F32, tag="dpart")
            nc.vector.tensor_reduce(
                out=dpart[:],
                in_=de_sb[:].rearrange("p (a j) -> p j a", j=6),
                axis=AX.X,
                op=ALU.add,
            )
            m_ps = pC.tile([6, 1], F32, tag="sm2")
            nc.tensor.matmul(m_ps[:], lhsT=dpart[:], rhs=ones32[:], start=True, stop=True)
            meanv = ep.tile([6, 1], F32, tag="meanv")
            nc.scalar.mul(meanv[:], m_ps[:], 1.0 / 1024.0)
            b2m = ep.tile([6, 1], F32, tag="b2m")
            nc.vector.tensor_tensor(out=b2m[:], in0=b2_sb[:], in1=meanv[:], op=ALU.mult)
            ovt_sb = ep.tile([6, SEGC], F32, tag="ovt")
            nc.scalar.activation(
                out=ovt_sb[:],
                in_=v_ps[:],
                func=AF.Identity,
                bias=b2m[:, :1],
                scale=meanv[:, :1],
            )
            nc.sync.dma_start(ovt[:], ovt_sb[:])

            # norm head
            hn_ps = pC.tile([P, SEGC], F32, tag="work")
            nc.tensor.matmul(hn_ps[:], lhsT=w1n_sb[:], rhs=xsT[:], start=True, stop=True)
            hnT = ep.tile([P, SEGC], F32, tag="hT")
            nc.scalar.activation(
                out=hnT[:], in_=hn_ps[:], func=AF.Relu, bias=b1n_sb[:, :1]
            )
            n_ps = pC.tile([1, SEGC], F32, tag="sm")
            nc.tensor.matmul(n_ps[:], lhsT=w2n_sb[:], rhs=hnT[:], start=True, stop=True)

            dnpart = ep.tile([P, 1], F32, tag="dnpart")
            nc.vector.tensor_reduce(out=dnpart[:], in_=dn_sb[:], axis=AX.X, op=ALU.add)
            mn_ps = pC.tile([1, 1], F32, tag="sm2")
            nc.tensor.matmul(
                mn_ps[:], lhsT=dnpart[:], rhs=ones32[:], start=True, stop=True
            )
            meann = ep.tile([1, 1], F32, tag="meann")
            nc.scalar.mul(meann[:], mn_ps[:], 1.0 / 1024.0)
            b2nm = ep.tile([1, 1], F32, tag="b2nm")
            nc.vector.tensor_tensor(
                out=b2nm[:], in0=b2n_sb[:], in1=meann[:], op=ALU.mult
            )
            ont_sb = ep.tile([1, SEGC], F32, tag="ont")
            nc.scalar.activation(
                out=ont_sb[:],
                in_=n_ps[:],
                func=AF.Identity,
                bias=b2nm[:1, :1],
                scale=meann[:1, :1],
            )
            nc.sync.dma_start(ont[:], ont_sb[:])

    return nc


def prepare_inputs(x_clu, mask_clu, batch_clu, dist_embedding, dist_embedding_norm,
                   gate_w, gate_b, w1, b1, w2, b2, w1n, b1n, w2n, b2n):
    """Host-side sharding / layout. Returns (NB, in_maps)."""
    x = np.asarray(x_clu, dtype=np.float32)
    mask = np.asarray(mask_clu).astype(bool)
    seg = np.asarray(batch_clu).astype(np.int64)
    N = x.shape[0]

    bounds = np.searchsorted(seg, np.arange(0, B_SEG + 1, SEGC))
    cnts = np.diff(bounds)
    NB = int(np.ceil(cnts.max() / NPB))
    NB = ((NB + BPG - 1) // BPG) * BPG
    NG = NB // BPG
    NTOT = NB * NPB

    # shared (replicated) small tensors
    shared = {
        "wrep": np.ascontiguousarray(
            np.broadcast_to(
                np.asarray(gate_w, np.float32).reshape(-1)[None, :], (P, D)
            ).astype(np.float16)
        ),
        "w1h": np.ascontiguousarray(np.asarray(w1, np.float32)),
        "b1h": np.asarray(b1, np.float32).reshape(D, 1).copy(),
        "w2h": np.ascontiguousarray(np.asarray(w2, np.float32)),
        "b2h": np.asarray(b2, np.float32).reshape(6, 1).copy(),
        "w1nh": np.ascontiguousarray(np.asarray(w1n, np.float32)),
        "b1nh": np.asarray(b1n, np.float32).reshape(D, 1).copy(),
        "w2nh": np.ascontiguousarray(np.asarray(w2n, np.float32)),
        "b2nh": np.asarray(b2n, np.float32).reshape(1, 1).copy(),
        "deh": np.asarray(dist_embedding, np.float32).reshape(P, 48).copy(),
        "dnh": np.asarray(dist_embedding_norm, np.float32).reshape(P, 8).copy(),
    }

    in_maps = []
    for c in range(NCORES):
        lo, hi = int(bounds[c]), int(bounds[c + 1])
        cnt = hi - lo

        xp = np.zeros((NTOT, D), np.float16)
        xp[:cnt] = x[lo:hi].astype(np.float16)
        xh = np.ascontiguousarray(
            xp.reshape(NB, TPB, P, D).transpose(0, 2, 1, 3)
        )

        rel = np.full(NTOT, -10_000, np.int64)
        rel[:cnt] = seg[lo:hi] - c * SEGC
        mk = np.zeros(NTOT, bool)
        mk[:cnt] = mask[lo:hi]

        relb = rel.reshape(NB, NPB)
        has_real = (relb >= 0).any(axis=1)
        LO = np.where(has_real, relb[:, 0], 0)
        # window span check (sorted ids -> max is at the last real node)
        span = np.where(has_real, relb.max(axis=1) - LO + 1, 1)
        assert span.max() <= WB, f"segment window {span.max()} exceeds WB={WB}"

        ohf = (
            (rel.reshape(NB, NPB, 1) - LO[:, None, None])
            == np.arange(WB)[None, None, :]
        ) & mk.reshape(NB, NPB, 1)
        ohh = np.ascontiguousarray(
            ohf.reshape(NB, TPB, P, WB).transpose(0, 2, 1, 3).astype(np.float16)
        )

        F = np.zeros((NG, P, SEGC), np.float16)
        for b in range(NB):
            if not has_real[b]:
                continue
            g, bb = divmod(b, BPG)
            js = np.arange(WB)
            ss = LO[b] + js
            valid = (ss >= 0) & (ss < SEGC)
            F[g, bb * GSTRIDE + js[valid], ss[valid]] = 1.0

        m = {"xh": xh, "ohh": ohh, "fh": F}
        m.update(shared)
        in_maps.append(m)

    return NB, bounds, in_maps


def _run(inputs: dict, trace: bool = False):
    NB, bounds, in_maps = prepare_inputs(**inputs)
    nc = build_program(NB)
    nc.finalize()
    res = run_bass_kernel_spmd(nc, in_maps, list(range(NCORES)), trace=trace)
    ov = np.concatenate([res.results[c]["ovt"].T for c in range(NCORES)], axis=0)
    on = np.concatenate([res.results[c]["ont"].T for c in range(NCORES)], axis=0)
    return (ov.astype(np.float32), on.astype(np.float32)), res


def kernel(**inputs):
    out, _ = _run(inputs, trace=False)
    return out


# revision 18
# speedup vs baseline: 1.0007x; 1.0007x over previous
"""Trainium2 Bass kernel for nn_Decoder (attention pooling over sorted segments + tiny MLPs).

Strategy (data-parallel over segments, 8 cores):
  - Core c owns segments [c*512, (c+1)*512) and the contiguous node range covering them
    (batch_clu is sorted, so each core's nodes are a contiguous slice).
  - Host pre-arranges per-core inputs:
      * x nodes cast to f16, laid out [block, partition(node%128), tile, d] so each DMA
        is contiguous per partition.
      * a windowed one-hot "assignment" matrix per block (window of WB=16 segments
        starting at the block's first segment), mask folded in, f16.
      * a fold matrix F mapping (block, window-col) staging slots -> the core's 512
        segments (applied on-device as a matmul at the end).
  - Device per block (2048 nodes = 16 tiles of 128):
      gate  = reduce_d(x_f16 * w_rep)           (DVE, f16 2x mode, two-stage reduce)
      e     = exp(gate)                         (ACT, f32 -> f16)
      won   = onehot * e                        (DVE)
      S^T  += won_t^T @ x_t   (PE, accumulates [WB x 128] per block into group PSUM)
      den  += won_t^T @ ones  (PE, [WB x 1])
    Groups of 8 blocks share one [128 x 128] PSUM tile (disjoint 16-partition slices),
    copied once per group to SBUF staging (f16).
  - Epilogue: fold staging -> per-segment sums via F matmuls, normalize by
    max(den,1e-12), run both MLP heads fully transposed (no PE transposes needed
    anywhere), scale by mean(dist_embedding), DMA out [6 x 512] + [1 x 512].
  - No max-subtraction in the segment softmax: softmax is shift-invariant and
    gate ~ N(0,1), so exp() is well-conditioned; gate bias cancels in the ratio.
"""

import sys

sys.path.insert(0, "/opt/trn_rl_repo")

import numpy as np
from contextlib import ExitStack

import concourse.bass as bass
import concourse.bacc as bacc
import concourse.mybir as mybir
import concourse.tile as tile
from concourse.bass_utils import run_bass_kernel_spmd

P = 128          # partitions / nodes per tile
D = 128          # feature dim
TPB = 16         # tiles per block
NPB = P * TPB    # nodes per block (2048)
WB = 16          # segment window width per block
BPG = 3          # blocks per PSUM group (PE out base partition must be 0/32/64)
GSTRIDE = 32     # partition stride between blocks within a group
NCORES = 8
B_SEG = 4096
SEGC = B_SEG // NCORES  # segments per core (512)

F32 = mybir.dt.float32
F16 = mybir.dt.float16
AX = mybir.AxisListType
ALU = mybir.AluOpType
AF = mybir.ActivationFunctionType


def build_program(NB: int):
    """Build the single SPMD Bass program (same for all 8 cores)."""
    assert NB % BPG == 0
    NG = NB // BPG

    # Bacc (not raw Bass): its compile() pass splits multi-sem waits into
    # event-semaphore chains — walrus rejects any instruction with >1 wait.
    nc = bacc.Bacc(None)

    # x features plus a packed ones-column (col D) so one matmul yields both
    # the weighted feature sums and the denominator row
    xh = nc.declare_dram_parameter("xh", [NB, P, TPB, D + 1], F16, isOutput=False)
    ohh = nc.declare_dram_parameter("ohh", [NB, P, TPB, WB], F16, isOutput=False)
    fh = nc.declare_dram_parameter("fh", [NG, P, SEGC], F16, isOutput=False)
    wrep = nc.declare_dram_parameter("wrep", [P, D], F16, isOutput=False)
    w1h = nc.declare_dram_parameter("w1h", [D, D], F32, isOutput=False)
    b1h = nc.declare_dram_parameter("b1h", [D, 1], F32, isOutput=False)
    w2h = nc.declare_dram_parameter("w2h", [D, 6], F32, isOutput=False)
    b2h = nc.declare_dram_parameter("b2h", [6, 1], F32, isOutput=False)
    w1nh = nc.declare_dram_parameter("w1nh", [D, D], F32, isOutput=False)
    b1nh = nc.declare_dram_parameter("b1nh", [D, 1], F32, isOutput=False)
    w2nh = nc.declare_dram_parameter("w2nh", [D, 1], F32, isOutput=False)
    b2nh = nc.declare_dram_parameter("b2nh", [1, 1], F32, isOutput=False)
    deh = nc.declare_dram_parameter("deh", [P, 48], F32, isOutput=False)
    dnh = nc.declare_dram_parameter("dnh", [P, 8], F32, isOutput=False)
    ovt = nc.declare_dram_parameter("ovt", [6, SEGC], F32, isOutput=True)
    ont = nc.declare_dram_parameter("ont", [1, SEGC], F32, isOutput=True)

    with tile.TileContext(nc) as tc, ExitStack() as ctx:
        cst = ctx.enter_context(tc.tile_pool(name="cst", bufs=1))

        wrep_sb = cst.tile([P, D], F16)
        nc.sync.dma_start(wrep_sb[:], wrep[:])
        w1_sb = cst.tile([D, D], F32)
        nc.sync.dma_start(w1_sb[:], w1h[:])
        b1_sb = cst.tile([D, 1], F32)
        nc.sync.dma_start(b1_sb[:], b1h[:])
        w2_sb = cst.tile([D, 6], F32)
        nc.sync.dma_start(w2_sb[:], w2h[:])
        b2_sb = cst.tile([6, 1], F32)
        nc.sync.dma_start(b2_sb[:], b2h[:])
        w1n_sb = cst.tile([D, D], F32)
        nc.sync.dma_start(w1n_sb[:], w1nh[:])
        b1n_sb = cst.tile([D, 1], F32)
        nc.sync.dma_start(b1n_sb[:], b1nh[:])
        w2n_sb = cst.tile([D, 1], F32)
        nc.sync.dma_start(w2n_sb[:], w2nh[:])
        b2n_sb = cst.tile([1, 1], F32)
        nc.sync.dma_start(b2n_sb[:], b2nh[:])
        de_sb = cst.tile([P, 48], F32)
        nc.sync.dma_start(de_sb[:], deh[:])
        dn_sb = cst.tile([P, 8], F32)
        nc.sync.dma_start(dn_sb[:], dnh[:])

        ones32 = cst.tile([P, 1], F32)
        nc.vector.memset(ones32[:], 1.0)
        ones_row = cst.tile([1, P], F32)
        nc.vector.memset(ones_row[:], 1.0)

        # staging for per-(block,window-col) partial sums
        stg = ctx.enter_context(tc.tile_pool(name="stg", bufs=1))
        stage_g = [
            stg.tile([P, D + 1], F16, tag=f"st{g}", name=f"stage{g}")
            for g in range(NG)
        ]

        with (
            tc.tile_pool(name="lp", bufs=3) as lp,
            tc.tile_pool(name="lps", bufs=2) as lps,
            tc.tile_pool(name="pl", bufs=2, space="PSUM") as pl,
        ):
            for g in range(NG):
                S_ps = pl.tile([P, D + 1], F32, tag="S")
                # blocks only cover partitions [32*bb, 32*bb+16); zero the rest
                nc.scalar.memzero(S_ps[:])
                for bb in range(BPG):
                    b = g * BPG + bb
                    x_sb = lp.tile([P, TPB, D + 1], F16, tag="x")
                    nc.sync.dma_start(x_sb[:], xh[b])
                    oh_sb = lp.tile([P, TPB, WB], F16, tag="oh")
                    nc.sync.dma_start(oh_sb[:], ohh[b])

                    prod = lps.tile([P, TPB, D], F16, tag="prod")
                    nc.vector.tensor_tensor(
                        out=prod[:],
                        in0=x_sb[:, :, :D],
                        in1=wrep_sb[:, None, :].to_broadcast([P, TPB, D]),
                        op=ALU.mult,
                    )
                    pg = lps.tile([P, TPB * 8], F16, tag="pg")
                    with nc.allow_low_precision(
                        reason="f16 partial sums of 16 terms; validated ~5e-4"
                    ):
                        nc.vector.tensor_reduce(
                            out=pg[:],
                            in_=prod[:].rearrange("p t d -> p (t d)").rearrange(
                                "p (tg e) -> p tg e", e=16
                            ),
                            axis=AX.X,
                            op=ALU.add,
                        )
                    gates = lps.tile([P, TPB], F32, tag="gates")
                    nc.vector.tensor_reduce(
                        out=gates[:],
                        in_=pg[:].rearrange("p (t g8) -> p t g8", g8=8),
                        axis=AX.X,
                        op=ALU.add,
                    )
                    e16 = lps.tile([P, TPB], F16, tag="e16")
                    nc.scalar.activation(out=e16[:], in_=gates[:], func=AF.Exp)
                    won = lps.tile([P, TPB, WB], F16, tag="won")
                    nc.vector.tensor_tensor(
                        out=won[:],
                        in0=oh_sb[:],
                        in1=e16[:, :, None].to_broadcast([P, TPB, WB]),
                        op=ALU.mult,
                    )
                    p0 = bb * GSTRIDE
                    for t in range(TPB):
                        nc.tensor.matmul(
                            S_ps[p0 : p0 + WB, :],
                            lhsT=won[:, t, :],
                            rhs=x_sb[:, t, :],
                            start=(t == 0),
                            stop=(t == TPB - 1),
                        )
                nc.scalar.copy(out=stage_g[g][:], in_=S_ps[:])

        # ---- epilogue ----
        with (
            tc.tile_pool(name="ep", bufs=2) as ep,
            tc.tile_pool(name="pA", bufs=1, space="PSUM") as pA,
            tc.tile_pool(name="pC", bufs=2, space="PSUM") as pC,
        ):
            S_acc = pA.tile([P, SEGC], F32, tag="Sacc")
            den_acc = pA.tile([1, SEGC], F32, tag="dacc")
            for g in range(NG):
                f_sb = ep.tile([P, SEGC], F16, tag="f")
                nc.sync.dma_start(f_sb[:], fh[g])
                nc.tensor.matmul(
                    S_acc[:],
                    lhsT=stage_g[g][:, :D],
                    rhs=f_sb[:],
                    start=(g == 0),
                    stop=(g == NG - 1),
                )
                nc.tensor.matmul(
                    den_acc[:],
                    lhsT=stage_g[g][:, D : D + 1],
                    rhs=f_sb[:],
                    start=(g == 0),
                    stop=(g == NG - 1),
                )

            dmax = ep.tile([1, SEGC], F32, tag="dmax")
            nc.vector.tensor_scalar_max(out=dmax[:], in0=den_acc[:], scalar1=1e-12)
            recip = ep.tile([1, SEGC], F32, tag="recip")
            nc.vector.reciprocal(recip[:], dmax[:])
            bc = pC.tile([P, SEGC], F32, tag="work")
            nc.tensor.matmul(bc[:], lhsT=ones_row[:], rhs=recip[:], start=True, stop=True)
            bc_sb = ep.tile([P, SEGC], F32, tag="bc_sb")
            nc.scalar.copy(out=bc_sb[:], in_=bc[:])
            xsT = ep.tile([P, SEGC], F32, tag="xsT")
            nc.vector.tensor_tensor(out=xsT[:], in0=S_acc[:], in1=bc_sb[:], op=ALU.mult)

            # vel head: hT = relu(w1^T xs^T + b1); v^T = w2^T hT; out = v^T*mean + b2*mean
            h_ps = pC.tile([P, SEGC], F32, tag="work")
            nc.tensor.matmul(h_ps[:], lhsT=w1_sb[:], rhs=xsT[:], start=True, stop=True)
            hT = ep.tile([P, SEGC], F32, tag="hT")
            nc.scalar.activation(out=hT[:], in_=h_ps[:], func=AF.Relu, bias=b1_sb[:, :1])
            v_ps = pC.tile([6, SEGC], F32, tag="sm")
            nc.tensor.matmul(v_ps[:], lhsT=w2_sb[:], rhs=hT[:], start=True, stop=True)

            dpart = ep.tile([P, 6], F32, tag="dpart")
            nc.vector.tensor_reduce(
                out=dpart[:],
                in_=de_sb[:].rearrange("p (a j) -> p j a", j=6),
                axis=AX.X,
                op=ALU.add,
            )
            m_ps = pC.tile([6, 1], F32, tag="sm2")
            nc.tensor.matmul(m_ps[:], lhsT=dpart[:], rhs=ones32[:], start=True, stop=True)
            meanv = ep.tile([6, 1], F32, tag="meanv")
            nc.scalar.mul(meanv[:], m_ps[:], 1.0 / 1024.0)
            b2m = ep.tile([6, 1], F32, tag="b2m")
            nc.vector.tensor_tensor(out=b2m[:], in0=b2_sb[:], in1=meanv[:], op=ALU.mult)
            ovt_sb = ep.tile([6, SEGC], F32, tag="ovt")
            nc.scalar.activation(
                out=ovt_sb[:],
                in_=v_ps[:],
                func=AF.Identity,
                bias=b2m[:, :1],
                scale=meanv[:, :1],
            )
            nc.sync.dma_start(ovt[:], ovt_sb[:])

            # norm head
            hn_ps = pC.tile([P, SEGC], F32, tag="work")
            nc.tensor.matmul(hn_ps[:], lhsT=w1n_sb[:], rhs=xsT[:], start=True, stop=True)
            hnT = ep.tile([P, SEGC], F32, tag="hT")
            nc.scalar.activation(
                out=hnT[:], in_=hn_ps[:], func=AF.Relu, bias=b1n_sb[:, :1]
            )
            n_ps = pC.tile([1, SEGC], F32, tag="sm")
            nc.tensor.matmul(n_ps[:], lhsT=w2n_sb[:], rhs=hnT[:], start=True, stop=True)

            dnpart = ep.tile([P, 1], F32, tag="dnpart")
            nc.vector.tensor_reduce(out=dnpart[:], in_=dn_sb[:], axis=AX.X, op=ALU.add)
            mn_ps = pC.tile([1, 1], F32, tag="sm2")
            nc.tensor.matmul(
                mn_ps[:], lhsT=dnpart[:], rhs=ones32[:], start=True, stop=True
            )
            meann = ep.tile([1, 1], F32, tag="meann")
            nc.scalar.mul(meann[:], mn_ps[:], 1.0 / 1024.0)
            b2nm = ep.tile([1, 1], F32, tag="b2nm")
            nc.vector.tensor_tensor(
                out=b2nm[:], in0=b2n_sb[:], in1=meann[:], op=ALU.mult
            )
            ont_sb = ep.tile([1, SEGC], F32, tag="ont")
            nc.scalar.activation(
                out=ont_sb[:],
                in_=n_ps[:],
                func=AF.Identity,
                bias=b2nm[:1, :1],
                scale=meann[:1, :1],
            )
            nc.sync.dma_start(ont[:], ont_sb[:])

    return nc


def prepare_inputs(x_clu, mask_clu, batch_clu, dist_embedding, dist_embedding_norm,
                   gate_w, gate_b, w1, b1, w2, b2, w1n, b1n, w2n, b2n):
    """Host-side sharding / layout. Returns (NB, in_maps)."""
    x = np.asarray(x_clu, dtype=np.float32)
    mask = np.asarray(mask_clu).astype(bool)
    seg = np.asarray(batch_clu).astype(np.int64)
    N = x.shape[0]

    bounds = np.searchsorted(seg, np.arange(0, B_SEG + 1, SEGC))
    cnts = np.diff(bounds)
    NB = int(np.ceil(cnts.max() / NPB))
    NB = ((NB + BPG - 1) // BPG) * BPG
    NG = NB // BPG
    NTOT = NB * NPB

    # shared (replicated) small tensors
    shared = {
        "wrep": np.ascontiguousarray(
            np.broadcast_to(
                np.asarray(gate_w, np.float32).reshape(-1)[None, :], (P, D)
            ).astype(np.float16)
        ),
        "w1h": np.ascontiguousarray(np.asarray(w1, np.float32)),
        "b1h": np.asarray(b1, np.float32).reshape(D, 1).copy(),
        "w2h": np.ascontiguousarray(np.asarray(w2, np.float32)),
        "b2h": np.asarray(b2, np.float32).reshape(6, 1).copy(),
        "w1nh": np.ascontiguousarray(np.asarray(w1n, np.float32)),
        "b1nh": np.asarray(b1n, np.float32).reshape(D, 1).copy(),
        "w2nh": np.ascontiguousarray(np.asarray(w2n, np.float32)),
        "b2nh": np.asarray(b2n, np.float32).reshape(1, 1).copy(),
        "deh": np.asarray(dist_embedding, np.float32).reshape(P, 48).copy(),
        "dnh": np.asarray(dist_embedding_norm, np.float32).reshape(P, 8).copy(),
    }

    in_maps = []
    for c in range(NCORES):
        lo, hi = int(bounds[c]), int(bounds[c + 1])
        cnt = hi - lo

        xp = np.zeros((NTOT, D + 1), np.float16)
        xp[:cnt, :D] = x[lo:hi].astype(np.float16)
        xp[:, D] = 1.0
        xh = np.ascontiguousarray(
            xp.reshape(NB, TPB, P, D + 1).transpose(0, 2, 1, 3)
        )

        rel = np.full(NTOT, -10_000, np.int64)
        rel[:cnt] = seg[lo:hi] - c * SEGC
        mk = np.zeros(NTOT, bool)
        mk[:cnt] = mask[lo:hi]

        relb = rel.reshape(NB, NPB)
        has_real = (relb >= 0).any(axis=1)
        LO = np.where(has_real, relb[:, 0], 0)
        # window span check (sorted ids -> max is at the last real node)
        span = np.where(has_real, relb.max(axis=1) - LO + 1, 1)
        assert span.max() <= WB, f"segment window {span.max()} exceeds WB={WB}"

        ohf = (
            (rel.reshape(NB, NPB, 1) - LO[:, None, None])
            == np.arange(WB)[None, None, :]
        ) & mk.reshape(NB, NPB, 1)
        ohh = np.ascontiguousarray(
            ohf.reshape(NB, TPB, P, WB).transpose(0, 2, 1, 3).astype(np.float16)
        )

        F = np.zeros((NG, P, SEGC), np.float16)
        for b in range(NB):
            if not has_real[b]:
                continue
            g, bb = divmod(b, BPG)
            js = np.arange(WB)
            ss = LO[b] + js
            valid = (ss >= 0) & (ss < SEGC)
            F[g, bb * GSTRIDE + js[valid], ss[valid]] = 1.0

        m = {"xh": xh, "ohh": ohh, "fh": F}
        m.update(shared)
        in_maps.append(m)

    return NB, bounds, in_maps


def _run(inputs: dict, trace: bool = False):
    NB, bounds, in_maps = prepare_inputs(**inputs)
    nc = build_program(NB)
    nc.finalize()
    res = run_bass_kernel_spmd(nc, in_maps, list(range(NCORES)), trace=trace)
    ov = np.concatenate([res.results[c]["ovt"].T for c in range(NCORES)], axis=0)
    on = np.concatenate([res.results[c]["ont"].T for c in range(NCORES)], axis=0)
    return (ov.astype(np.float32), on.astype(np.float32)), res


def kernel(**inputs):
    out, _ = _run(inputs, trace=False)
    return out


# revision 19
# speedup vs baseline: 1.1612x; 1.1604x over previous
"""Trainium2 Bass kernel for nn_Decoder (attention pooling over sorted segments + tiny MLPs).

Strategy (data-parallel over segments, 8 cores):
  - Core c owns segments [c*512, (c+1)*512) and the contiguous node range covering them
    (batch_clu is sorted, so each core's nodes are a contiguous slice).
  - Host pre-arranges per-core inputs:
      * x nodes cast to f16, laid out [block, partition(node%128), tile, d] so each DMA
        is contiguous per partition.
      * a windowed one-hot "assignment" matrix per block (window of WB=16 segments
        starting at the block's first segment), mask folded in, f16.
      * a fold matrix F mapping (block, window-col) staging slots -> the core's 512
        segments (applied on-device as a matmul at the end).
  - Device per block (2048 nodes = 16 tiles of 128):
      gate  = reduce_d(x_f16 * w_rep)           (DVE, f16 2x mode, two-stage reduce)
      e     = exp(gate)                         (ACT, f32 -> f16)
      won   = onehot * e                        (DVE)
      S^T  += won_t^T @ x_t   (PE, accumulates [WB x 128] per block into group PSUM)
      den  += won_t^T @ ones  (PE, [WB x 1])
    Groups of 8 blocks share one [128 x 128] PSUM tile (disjoint 16-partition slices),
    copied once per group to SBUF staging (f16).
  - Epilogue: fold staging -> per-segment sums via F matmuls, normalize by
    max(den,1e-12), run both MLP heads fully transposed (no PE transposes needed
    anywhere), scale by mean(dist_embedding), DMA out [6 x 512] + [1 x 512].
  - No max-subtraction in the segment softmax: softmax is shift-invariant and
    gate ~ N(0,1), so exp() is well-conditioned; gate bias cancels in the ratio.
"""

import sys

sys.path.insert(0, "/opt/trn_rl_repo")

import numpy as np
from contextlib import ExitStack

import concourse.bass as bass
import concourse.bacc as bacc
import concourse.mybir as mybir
import concourse.tile as tile
from concourse.bass_utils import run_bass_kernel_spmd

P = 128          # partitions / nodes per tile
D = 128          # feature dim
TPB = 16         # tiles per block
NPB = P * TPB    # nodes per block (2048)
WB = 16          # segment window width per block
BPG = 3          # blocks per PSUM group (PE out base partition must be 0/32/64)
GSTRIDE = 32     # partition stride between blocks within a group
NCORES = 8
B_SEG = 4096
SEGC = B_SEG // NCORES  # segments per core (512)

F32 = mybir.dt.float32
F16 = mybir.dt.float16
AX = mybir.AxisListType
ALU = mybir.AluOpType
AF = mybir.ActivationFunctionType


def build_program(NB: int):
    """Build the single SPMD Bass program (same for all 8 cores)."""
    assert NB % BPG == 0
    NG = NB // BPG

    # Bacc (not raw Bass): its compile() pass splits multi-sem waits into
    # event-semaphore chains — walrus rejects any instruction with >1 wait.
    nc = bacc.Bacc(None)

    # x features plus a packed ones-column (col D) so one matmul yields both
    # the weighted feature sums and the denominator row
    xh = nc.declare_dram_parameter("xh", [NB, P, TPB, D + 1], F16, isOutput=False)
    ohh = nc.declare_dram_parameter("ohh", [NB, P, TPB, WB], F16, isOutput=False)
    fh = nc.declare_dram_parameter("fh", [NG, P, SEGC], F16, isOutput=False)
    wrep = nc.declare_dram_parameter("wrep", [P, D], F16, isOutput=False)
    w1h = nc.declare_dram_parameter("w1h", [D, D], F32, isOutput=False)
    b1h = nc.declare_dram_parameter("b1h", [D, 1], F32, isOutput=False)
    w2h = nc.declare_dram_parameter("w2h", [D, 6], F32, isOutput=False)
    b2h = nc.declare_dram_parameter("b2h", [6, 1], F32, isOutput=False)
    w1nh = nc.declare_dram_parameter("w1nh", [D, D], F32, isOutput=False)
    b1nh = nc.declare_dram_parameter("b1nh", [D, 1], F32, isOutput=False)
    w2nh = nc.declare_dram_parameter("w2nh", [D, 1], F32, isOutput=False)
    b2nh = nc.declare_dram_parameter("b2nh", [1, 1], F32, isOutput=False)
    deh = nc.declare_dram_parameter("deh", [P, 48], F32, isOutput=False)
    dnh = nc.declare_dram_parameter("dnh", [P, 8], F32, isOutput=False)
    ovt = nc.declare_dram_parameter("ovt", [6, SEGC], F32, isOutput=True)
    ont = nc.declare_dram_parameter("ont", [1, SEGC], F32, isOutput=True)

    with tile.TileContext(nc) as tc, ExitStack() as ctx:
        cst = ctx.enter_context(tc.tile_pool(name="cst", bufs=1))

        wrep_sb = cst.tile([P, D], F16)
        nc.sync.dma_start(wrep_sb[:], wrep[:])

        # staging for per-(block,window-col) partial sums
        stg = ctx.enter_context(tc.tile_pool(name="stg", bufs=1))
        stage_g = [
            stg.tile([P, D + 1], F16, tag=f"st{g}", name=f"stage{g}")
            for g in range(NG)
        ]

        with (
            tc.tile_pool(name="lp", bufs=6) as lp,
            tc.tile_pool(name="lps", bufs=4) as lps,
            tc.tile_pool(name="pl", bufs=3, space="PSUM") as pl,
        ):
            # one-block software pipeline: emit won(b-1)+matmuls(b-1) after
            # exp(b) is issued, so the DVE never stalls on the ACT exp latency
            S_tiles = {}
            pending = None  # (b, x_sb, oh_sb, e16)

            def flush(pend):
                b, x_sb, oh_sb, e16 = pend
                g, bb = divmod(b, BPG)
                S_ps = S_tiles[g]
                won = lps.tile([P, TPB, WB], F16, tag="won", name=f"won{b}")
                nc.vector.tensor_tensor(
                    out=won[:],
                    in0=oh_sb[:],
                    in1=e16[:, :, None].to_broadcast([P, TPB, WB]),
                    op=ALU.mult,
                )
                p0 = bb * GSTRIDE
                for t in range(TPB):
                    nc.tensor.matmul(
                        S_ps[p0 : p0 + WB, :],
                        lhsT=won[:, t, :],
                        rhs=x_sb[:, t, :],
                        start=(t == 0),
                        stop=(t == TPB - 1),
                    )
                if bb == BPG - 1:
                    nc.scalar.copy(out=stage_g[g][:], in_=S_ps[:])
                    del S_tiles[g]

            for b in range(NB):
                g, bb = divmod(b, BPG)
                if bb == 0:
                    S_ps = pl.tile([P, D + 1], F32, tag="S", name=f"S{g}")
                    # blocks cover partitions [32*bb, 32*bb+16); zero the rest
                    nc.scalar.memzero(S_ps[:])
                    S_tiles[g] = S_ps
                x_sb = lp.tile([P, TPB, D + 1], F16, tag="x", name=f"x{b}")
                nc.sync.dma_start(x_sb[:], xh[b])
                oh_sb = lp.tile([P, TPB, WB], F16, tag="oh", name=f"oh{b}")
                nc.sync.dma_start(oh_sb[:], ohh[b])

                prod = lps.tile([P, TPB, D], F16, tag="prod", name=f"prod{b}")
                nc.vector.tensor_tensor(
                    out=prod[:],
                    in0=x_sb[:, :, :D],
                    in1=wrep_sb[:, None, :].to_broadcast([P, TPB, D]),
                    op=ALU.mult,
                )
                pg = lps.tile([P, TPB * 8], F16, tag="pg", name=f"pg{b}")
                with nc.allow_low_precision(
                    reason="f16 partial sums of 16 terms; validated ~5e-4"
                ):
                    nc.vector.tensor_reduce(
                        out=pg[:],
                        in_=prod[:].rearrange("p t d -> p (t d)").rearrange(
                            "p (tg e) -> p tg e", e=16
                        ),
                        axis=AX.X,
                        op=ALU.add,
                    )
                gates = lps.tile([P, TPB], F32, tag="gates", name=f"gates{b}")
                nc.vector.tensor_reduce(
                    out=gates[:],
                    in_=pg[:].rearrange("p (t g8) -> p t g8", g8=8),
                    axis=AX.X,
                    op=ALU.add,
                )
                e16 = lps.tile([P, TPB], F16, tag="e16", name=f"e16{b}")
                nc.scalar.activation(out=e16[:], in_=gates[:], func=AF.Exp)
                if pending is not None:
                    flush(pending)
                pending = (b, x_sb, oh_sb, e16)
            flush(pending)

        # epilogue-only constants (loaded late so startup DMA is not serialized)
        w1_sb = cst.tile([D, D], F32)
        nc.sync.dma_start(w1_sb[:], w1h[:])
        b1_sb = cst.tile([D, 1], F32)
        nc.sync.dma_start(b1_sb[:], b1h[:])
        w2_sb = cst.tile([D, 6], F32)
        nc.sync.dma_start(w2_sb[:], w2h[:])
        b2_sb = cst.tile([6, 1], F32)
        nc.sync.dma_start(b2_sb[:], b2h[:])
        w1n_sb = cst.tile([D, D], F32)
        nc.sync.dma_start(w1n_sb[:], w1nh[:])
        b1n_sb = cst.tile([D, 1], F32)
        nc.sync.dma_start(b1n_sb[:], b1nh[:])
        w2n_sb = cst.tile([D, 1], F32)
        nc.sync.dma_start(w2n_sb[:], w2nh[:])
        b2n_sb = cst.tile([1, 1], F32)
        nc.sync.dma_start(b2n_sb[:], b2nh[:])
        de_sb = cst.tile([P, 48], F32)
        nc.sync.dma_start(de_sb[:], deh[:])
        dn_sb = cst.tile([P, 8], F32)
        nc.sync.dma_start(dn_sb[:], dnh[:])

        ones32 = cst.tile([P, 1], F32)
        nc.vector.memset(ones32[:], 1.0)
        ones_row = cst.tile([1, P], F32)
        nc.vector.memset(ones_row[:], 1.0)

        # ---- epilogue ----
        with (
            tc.tile_pool(name="ep", bufs=2) as ep,
            tc.tile_pool(name="pA", bufs=1, space="PSUM") as pA,
            tc.tile_pool(name="pC", bufs=2, space="PSUM") as pC,
        ):
            S_acc = pA.tile([P, SEGC], F32, tag="Sacc")
            den_acc = pA.tile([1, SEGC], F32, tag="dacc")
            for g in range(NG):
                f_sb = ep.tile([P, SEGC], F16, tag="f")
                nc.sync.dma_start(f_sb[:], fh[g])
                nc.tensor.matmul(
                    S_acc[:],
                    lhsT=stage_g[g][:, :D],
                    rhs=f_sb[:],
                    start=(g == 0),
                    stop=(g == NG - 1),
                )
                nc.tensor.matmul(
                    den_acc[:],
                    lhsT=stage_g[g][:, D : D + 1],
                    rhs=f_sb[:],
                    start=(g == 0),
                    stop=(g == NG - 1),
                )

            dmax = ep.tile([1, SEGC], F32, tag="dmax")
            nc.vector.tensor_scalar_max(out=dmax[:], in0=den_acc[:], scalar1=1e-12)
            recip = ep.tile([1, SEGC], F32, tag="recip")
            nc.vector.reciprocal(recip[:], dmax[:])
            bc = pC.tile([P, SEGC], F32, tag="work")
            nc.tensor.matmul(bc[:], lhsT=ones_row[:], rhs=recip[:], start=True, stop=True)
            bc_sb = ep.tile([P, SEGC], F32, tag="bc_sb")
            nc.scalar.copy(out=bc_sb[:], in_=bc[:])
            xsT = ep.tile([P, SEGC], F32, tag="xsT")
            nc.vector.tensor_tensor(out=xsT[:], in0=S_acc[:], in1=bc_sb[:], op=ALU.mult)

            # vel head: hT = relu(w1^T xs^T + b1); v^T = w2^T hT; out = v^T*mean + b2*mean
            h_ps = pC.tile([P, SEGC], F32, tag="work")
            nc.tensor.matmul(h_ps[:], lhsT=w1_sb[:], rhs=xsT[:], start=True, stop=True)
            hT = ep.tile([P, SEGC], F32, tag="hT")
            nc.scalar.activation(out=hT[:], in_=h_ps[:], func=AF.Relu, bias=b1_sb[:, :1])
            v_ps = pC.tile([6, SEGC], F32, tag="sm")
            nc.tensor.matmul(v_ps[:], lhsT=w2_sb[:], rhs=hT[:], start=True, stop=True)

            dpart = ep.tile([P, 6], F32, tag="dpart")
            nc.vector.tensor_reduce(
                out=dpart[:],
                in_=de_sb[:].rearrange("p (a j) -> p j a", j=6),
                axis=AX.X,
                op=ALU.add,
            )
            m_ps = pC.tile([6, 1], F32, tag="sm2")
            nc.tensor.matmul(m_ps[:], lhsT=dpart[:], rhs=ones32[:], start=True, stop=True)
            meanv = ep.tile([6, 1], F32, tag="meanv")
            nc.scalar.mul(meanv[:], m_ps[:], 1.0 / 1024.0)
            b2m = ep.tile([6, 1], F32, tag="b2m")
            nc.vector.tensor_tensor(out=b2m[:], in0=b2_sb[:], in1=meanv[:], op=ALU.mult)
            ovt_sb = ep.tile([6, SEGC], F32, tag="ovt")
            nc.scalar.activation(
                out=ovt_sb[:],
                in_=v_ps[:],
                func=AF.Identity,
                bias=b2m[:, :1],
                scale=meanv[:, :1],
            )
            nc.sync.dma_start(ovt[:], ovt_sb[:])

            # norm head
            hn_ps = pC.tile([P, SEGC], F32, tag="work")
            nc.tensor.matmul(hn_ps[:], lhsT=w1n_sb[:], rhs=xsT[:], start=True, stop=True)
            hnT = ep.tile([P, SEGC], F32, tag="hT")
            nc.scalar.activation(
                out=hnT[:], in_=hn_ps[:], func=AF.Relu, bias=b1n_sb[:, :1]
            )
            n_ps = pC.tile([1, SEGC], F32, tag="sm")
            nc.tensor.matmul(n_ps[:], lhsT=w2n_sb[:], rhs=hnT[:], start=True, stop=True)

            dnpart = ep.tile([P, 1], F32, tag="dnpart")
            nc.vector.tensor_reduce(out=dnpart[:], in_=dn_sb[:], axis=AX.X, op=ALU.add)
            mn_ps = pC.tile([1, 1], F32, tag="sm2")
            nc.tensor.matmul(
                mn_ps[:], lhsT=dnpart[:], rhs=ones32[:], start=True, stop=True
            )
            meann = ep.tile([1, 1], F32, tag="meann")
            nc.scalar.mul(meann[:], mn_ps[:], 1.0 / 1024.0)
            b2nm = ep.tile([1, 1], F32, tag="b2nm")
            nc.vector.tensor_tensor(
                out=b2nm[:], in0=b2n_sb[:], in1=meann[:], op=ALU.mult
            )
            ont_sb = ep.tile([1, SEGC], F32, tag="ont")
            nc.scalar.activation(
                out=ont_sb[:],
                in_=n_ps[:],
                func=AF.Identity,
                bias=b2nm[:1, :1],
                scale=meann[:1, :1],
            )
            nc.sync.dma_start(ont[:], ont_sb[:])

    return nc


def prepare_inputs(x_clu, mask_clu, batch_clu, dist_embedding, dist_embedding_norm,
                   gate_w, gate_b, w1, b1, w2, b2, w1n, b1n, w2n, b2n):
    """Host-side sharding / layout. Returns (NB, in_maps)."""
    x = np.asarray(x_clu, dtype=np.float32)
    mask = np.asarray(mask_clu).astype(bool)
    seg = np.asarray(batch_clu).astype(np.int64)
    N = x.shape[0]

    bounds = np.searchsorted(seg, np.arange(0, B_SEG + 1, SEGC))
    cnts = np.diff(bounds)
    NB = int(np.ceil(cnts.max() / NPB))
    NB = ((NB + BPG - 1) // BPG) * BPG
    NG = NB // BPG
    NTOT = NB * NPB

    # shared (replicated) small tensors
    shared = {
        "wrep": np.ascontiguousarray(
            np.broadcast_to(
                np.asarray(gate_w, np.float32).reshape(-1)[None, :], (P, D)
            ).astype(np.float16)
        ),
        "w1h": np.ascontiguousarray(np.asarray(w1, np.float32)),
        "b1h": np.asarray(b1, np.float32).reshape(D, 1).copy(),
        "w2h": np.ascontiguousarray(np.asarray(w2, np.float32)),
        "b2h": np.asarray(b2, np.float32).reshape(6, 1).copy(),
        "w1nh": np.ascontiguousarray(np.asarray(w1n, np.float32)),
        "b1nh": np.asarray(b1n, np.float32).reshape(D, 1).copy(),
        "w2nh": np.ascontiguousarray(np.asarray(w2n, np.float32)),
        "b2nh": np.asarray(b2n, np.float32).reshape(1, 1).copy(),
        "deh": np.asarray(dist_embedding, np.float32).reshape(P, 48).copy(),
        "dnh": np.asarray(dist_embedding_norm, np.float32).reshape(P, 8).copy(),
    }

    in_maps = []
    for c in range(NCORES):
        lo, hi = int(bounds[c]), int(bounds[c + 1])
        cnt = hi - lo

        xp = np.zeros((NTOT, D + 1), np.float16)
        xp[:cnt, :D] = x[lo:hi].astype(np.float16)
        xp[:, D] = 1.0
        xh = np.ascontiguousarray(
            xp.reshape(NB, TPB, P, D + 1).transpose(0, 2, 1, 3)
        )

        rel = np.full(NTOT, -10_000, np.int64)
        rel[:cnt] = seg[lo:hi] - c * SEGC
        mk = np.zeros(NTOT, bool)
        mk[:cnt] = mask[lo:hi]

        relb = rel.reshape(NB, NPB)
        has_real = (relb >= 0).any(axis=1)
        LO = np.where(has_real, relb[:, 0], 0)
        # window span check (sorted ids -> max is at the last real node)
        span = np.where(has_real, relb.max(axis=1) - LO + 1, 1)
        assert span.max() <= WB, f"segment window {span.max()} exceeds WB={WB}"

        ohf = (
            (rel.reshape(NB, NPB, 1) - LO[:, None, None])
            == np.arange(WB)[None, None, :]
        ) & mk.reshape(NB, NPB, 1)
        ohh = np.ascontiguousarray(
            ohf.reshape(NB, TPB, P, WB).transpose(0, 2, 1, 3).astype(np.float16)
        )

        F = np.zeros((NG, P, SEGC), np.float16)
        for b in range(NB):
            if not has_real[b]:
                continue
            g, bb = divmod(b, BPG)
            js = np.arange(WB)
            ss = LO[b] + js
            valid = (ss >= 0) & (ss < SEGC)
            F[g, bb * GSTRIDE + js[valid], ss[valid]] = 1.0

        m = {"xh": xh, "ohh": ohh, "fh": F}
        m.update(shared)
        in_maps.append(m)

    return NB, bounds, in_maps


def _run(inputs: dict, trace: bool = False):
    NB, bounds, in_maps = prepare_inputs(**inputs)
    nc = build_program(NB)
    nc.finalize()
    res = run_bass_kernel_spmd(nc, in_maps, list(range(NCORES)), trace=trace)
    ov = np.concatenate([res.results[c]["ovt"].T for c in range(NCORES)], axis=0)
    on = np.concatenate([res.results[c]["ont"].T for c in range(NCORES)], axis=0)
    return (ov.astype(np.float32), on.astype(np.float32)), res


def kernel(**inputs):
    out, _ = _run(inputs, trace=False)
    return out


# revision 20
# speedup vs baseline: 1.4357x; 1.2365x over previous
"""Trainium2 Bass kernel for nn_Decoder (attention pooling over sorted segments + tiny MLPs).

Strategy (data-parallel over segments, 8 cores):
  - Core c owns segments [c*512, (c+1)*512) and the contiguous node range covering them
    (batch_clu is sorted, so each core's nodes are a contiguous slice).
  - Host pre-arranges per-core inputs:
      * x nodes cast to f16, laid out [block, partition(node%128), tile, d] so each DMA
        is contiguous per partition.
      * a windowed one-hot "assignment" matrix per block (window of WB=16 segments
        starting at the block's first segment), mask folded in, f16.
      * a fold matrix F mapping (block, window-col) staging slots -> the core's 512
        segments (applied on-device as a matmul at the end).
  - Device per block (2048 nodes = 16 tiles of 128):
      gate  = reduce_d(x_f16 * w_rep)           (DVE, f16 2x mode, two-stage reduce)
      e     = exp(gate)                         (ACT, f32 -> f16)
      won   = onehot * e                        (DVE)
      S^T  += won_t^T @ x_t   (PE, accumulates [WB x 128] per block into group PSUM)
      den  += won_t^T @ ones  (PE, [WB x 1])
    Groups of 8 blocks share one [128 x 128] PSUM tile (disjoint 16-partition slices),
    copied once per group to SBUF staging (f16).
  - Epilogue: fold staging -> per-segment sums via F matmuls, normalize by
    max(den,1e-12), run both MLP heads fully transposed (no PE transposes needed
    anywhere), scale by mean(dist_embedding), DMA out [6 x 512] + [1 x 512].
  - No max-subtraction in the segment softmax: softmax is shift-invariant and
    gate ~ N(0,1), so exp() is well-conditioned; gate bias cancels in the ratio.
"""

import sys

sys.path.insert(0, "/opt/trn_rl_repo")

import numpy as np
from contextlib import ExitStack

import concourse.bass as bass
import concourse.bacc as bacc
import concourse.mybir as mybir
import concourse.tile as tile
from concourse.bass_utils import run_bass_kernel_spmd

P = 128          # partitions / nodes per tile
D = 128          # feature dim
TPB = 16         # tiles per block
NPB = P * TPB    # nodes per block (2048)
WB = 16          # segment window width per block
BPG = 3          # blocks per PSUM group (PE out base partition must be 0/32/64)
GSTRIDE = 32     # partition stride between blocks within a group
NCORES = 8
B_SEG = 4096
SEGC = B_SEG // NCORES  # segments per core (512)

F32 = mybir.dt.float32
F16 = mybir.dt.float16
AX = mybir.AxisListType
ALU = mybir.AluOpType
AF = mybir.ActivationFunctionType


def build_program(NB: int):
    """Build the single SPMD Bass program (same for all 8 cores)."""
    assert NB % BPG == 0
    NG = NB // BPG

    # Bacc (not raw Bass): its compile() pass splits multi-sem waits into
    # event-semaphore chains — walrus rejects any instruction with >1 wait.
    nc = bacc.Bacc(None)

    # x features plus a packed ones-column (col D) so one matmul yields both
    # the weighted feature sums and the denominator row
    xh = nc.declare_dram_parameter("xh", [NB, P, TPB, D + 1], F16, isOutput=False)
    ohh = nc.declare_dram_parameter("ohh", [NB, P, TPB, WB], F16, isOutput=False)
    fh = nc.declare_dram_parameter("fh", [NG, P, SEGC], F16, isOutput=False)
    wrep = nc.declare_dram_parameter("wrep", [P, D], F16, isOutput=False)
    w1h = nc.declare_dram_parameter("w1h", [D, D], F32, isOutput=False)
    b1h = nc.declare_dram_parameter("b1h", [D, 1], F32, isOutput=False)
    w2h = nc.declare_dram_parameter("w2h", [D, 6], F32, isOutput=False)
    b2h = nc.declare_dram_parameter("b2h", [6, 1], F32, isOutput=False)
    w1nh = nc.declare_dram_parameter("w1nh", [D, D], F32, isOutput=False)
    b1nh = nc.declare_dram_parameter("b1nh", [D, 1], F32, isOutput=False)
    w2nh = nc.declare_dram_parameter("w2nh", [D, 1], F32, isOutput=False)
    b2nh = nc.declare_dram_parameter("b2nh", [1, 1], F32, isOutput=False)
    deh = nc.declare_dram_parameter("deh", [P, 48], F32, isOutput=False)
    dnh = nc.declare_dram_parameter("dnh", [P, 8], F32, isOutput=False)
    ovt = nc.declare_dram_parameter("ovt", [6, SEGC], F32, isOutput=True)
    ont = nc.declare_dram_parameter("ont", [1, SEGC], F32, isOutput=True)

    with tile.TileContext(nc) as tc, ExitStack() as ctx:
        cst = ctx.enter_context(tc.tile_pool(name="cst", bufs=1))

        wrep_sb = cst.tile([P, D], F16)
        nc.sync.dma_start(wrep_sb[:], wrep[:])

        # staging for per-(block,window-col) partial sums
        stg = ctx.enter_context(tc.tile_pool(name="stg", bufs=1))
        stage_g = [
            stg.tile([P, D + 1], F16, tag=f"st{g}", name=f"stage{g}")
            for g in range(NG)
        ]

        with (
            tc.tile_pool(name="lp", bufs=6) as lp,
            tc.tile_pool(name="lps", bufs=4) as lps,
            tc.tile_pool(name="pl", bufs=3, space="PSUM") as pl,
        ):
            # one-block software pipeline: emit won(b-1)+matmuls(b-1) after
            # exp(b) is issued, so the DVE never stalls on the ACT exp latency
            S_tiles = {}
            pending = None  # (b, x_sb, oh_sb, e16)

            def flush(pend):
                b, x_sb, oh_sb, e16 = pend
                g, bb = divmod(b, BPG)
                S_ps = S_tiles[g]
                won = lps.tile([P, TPB, WB], F16, tag="won", name=f"won{b}")
                nc.vector.tensor_tensor(
                    out=won[:],
                    in0=oh_sb[:],
                    in1=e16[:, :, None].to_broadcast([P, TPB, WB]),
                    op=ALU.mult,
                )
                p0 = bb * GSTRIDE
                for t in range(TPB):
                    nc.tensor.matmul(
                        S_ps[p0 : p0 + WB, :],
                        lhsT=won[:, t, :],
                        rhs=x_sb[:, t, :],
                        start=(t == 0),
                        stop=(t == TPB - 1),
                    )
                if bb == BPG - 1:
                    nc.scalar.copy(out=stage_g[g][:], in_=S_ps[:])
                    del S_tiles[g]

            for b in range(NB):
                g, bb = divmod(b, BPG)
                if bb == 0:
                    S_ps = pl.tile([P, D + 1], F32, tag="S", name=f"S{g}")
                    # blocks cover partitions [32*bb, 32*bb+16); zero the rest
                    nc.scalar.memzero(S_ps[:])
                    S_tiles[g] = S_ps
                x_sb = lp.tile([P, TPB, D + 1], F16, tag="x", name=f"x{b}")
                nc.sync.dma_start(x_sb[:], xh[b])
                oh_sb = lp.tile([P, TPB, WB], F16, tag="oh", name=f"oh{b}")
                nc.sync.dma_start(oh_sb[:], ohh[b])

                prod = lps.tile([P, TPB, D], F16, tag="prod", name=f"prod{b}")
                nc.vector.tensor_tensor(
                    out=prod[:],
                    in0=x_sb[:, :, :D],
                    in1=wrep_sb[:, None, :].to_broadcast([P, TPB, D]),
                    op=ALU.mult,
                )
                # pairwise f16 tree (tensor_tensor adds run in 2x mode; a
                # single tensor_reduce would fall back to 1x), then one small
                # fp32 reduce over the remaining 16 partials per tile
                pv = prod[:].rearrange("p t d -> p (t d)").rearrange(
                    "p (tg e) -> p tg e", e=16
                )
                a1 = lps.tile([P, 128, 8], F16, tag="a1", name=f"a1{b}")
                a2 = lps.tile([P, 128, 4], F16, tag="a2", name=f"a2{b}")
                a3 = lps.tile([P, 128, 2], F16, tag="a3", name=f"a3{b}")
                with nc.allow_low_precision(
                    reason="f16 pairwise partial sums; validated ~5e-4"
                ):
                    nc.vector.tensor_tensor(
                        out=a1[:], in0=pv[:, :, 0:8], in1=pv[:, :, 8:16], op=ALU.add
                    )
                    nc.vector.tensor_tensor(
                        out=a2[:], in0=a1[:, :, 0:4], in1=a1[:, :, 4:8], op=ALU.add
                    )
                    nc.vector.tensor_tensor(
                        out=a3[:], in0=a2[:, :, 0:2], in1=a2[:, :, 2:4], op=ALU.add
                    )
                gates = lps.tile([P, TPB], F32, tag="gates", name=f"gates{b}")
                nc.vector.tensor_reduce(
                    out=gates[:],
                    in_=a3[:].rearrange("p (t g8) two -> p t (g8 two)", g8=8),
                    axis=AX.X,
                    op=ALU.add,
                )
                e16 = lps.tile([P, TPB], F16, tag="e16", name=f"e16{b}")
                nc.scalar.activation(out=e16[:], in_=gates[:], func=AF.Exp)
                if pending is not None:
                    flush(pending)
                pending = (b, x_sb, oh_sb, e16)
            flush(pending)

        # epilogue-only constants (loaded late so startup DMA is not serialized)
        w1_sb = cst.tile([D, D], F32)
        nc.sync.dma_start(w1_sb[:], w1h[:])
        b1_sb = cst.tile([D, 1], F32)
        nc.sync.dma_start(b1_sb[:], b1h[:])
        w2_sb = cst.tile([D, 6], F32)
        nc.sync.dma_start(w2_sb[:], w2h[:])
        b2_sb = cst.tile([6, 1], F32)
        nc.sync.dma_start(b2_sb[:], b2h[:])
        w1n_sb = cst.tile([D, D], F32)
        nc.sync.dma_start(w1n_sb[:], w1nh[:])
        b1n_sb = cst.tile([D, 1], F32)
        nc.sync.dma_start(b1n_sb[:], b1nh[:])
        w2n_sb = cst.tile([D, 1], F32)
        nc.sync.dma_start(w2n_sb[:], w2nh[:])
        b2n_sb = cst.tile([1, 1], F32)
        nc.sync.dma_start(b2n_sb[:], b2nh[:])
        de_sb = cst.tile([P, 48], F32)
        nc.sync.dma_start(de_sb[:], deh[:])
        dn_sb = cst.tile([P, 8], F32)
        nc.sync.dma_start(dn_sb[:], dnh[:])

        ones32 = cst.tile([P, 1], F32)
        nc.vector.memset(ones32[:], 1.0)
        ones_row = cst.tile([1, P], F32)
        nc.vector.memset(ones_row[:], 1.0)

        # ---- epilogue ----
        with (
            tc.tile_pool(name="ep", bufs=2) as ep,
            tc.tile_pool(name="pA", bufs=1, space="PSUM") as pA,
            tc.tile_pool(name="pC", bufs=2, space="PSUM") as pC,
        ):
            S_acc = pA.tile([P, SEGC], F32, tag="Sacc")
            den_acc = pA.tile([1, SEGC], F32, tag="dacc")
            for g in range(NG):
                f_sb = ep.tile([P, SEGC], F16, tag="f")
                nc.sync.dma_start(f_sb[:], fh[g])
                nc.tensor.matmul(
                    S_acc[:],
                    lhsT=stage_g[g][:, :D],
                    rhs=f_sb[:],
                    start=(g == 0),
                    stop=(g == NG - 1),
                )
                nc.tensor.matmul(
                    den_acc[:],
                    lhsT=stage_g[g][:, D : D + 1],
                    rhs=f_sb[:],
                    start=(g == 0),
                    stop=(g == NG - 1),
                )

            dmax = ep.tile([1, SEGC], F32, tag="dmax")
            nc.vector.tensor_scalar_max(out=dmax[:], in0=den_acc[:], scalar1=1e-12)
            recip = ep.tile([1, SEGC], F32, tag="recip")
            nc.vector.reciprocal(recip[:], dmax[:])
            bc = pC.tile([P, SEGC], F32, tag="work")
            nc.tensor.matmul(bc[:], lhsT=ones_row[:], rhs=recip[:], start=True, stop=True)
            bc_sb = ep.tile([P, SEGC], F32, tag="bc_sb")
            nc.scalar.copy(out=bc_sb[:], in_=bc[:])
            xsT = ep.tile([P, SEGC], F32, tag="xsT")
            nc.vector.tensor_tensor(out=xsT[:], in0=S_acc[:], in1=bc_sb[:], op=ALU.mult)

            # vel head: hT = relu(w1^T xs^T + b1); v^T = w2^T hT; out = v^T*mean + b2*mean
            h_ps = pC.tile([P, SEGC], F32, tag="work")
            nc.tensor.matmul(h_ps[:], lhsT=w1_sb[:], rhs=xsT[:], start=True, stop=True)
            hT = ep.tile([P, SEGC], F32, tag="hT")
            nc.scalar.activation(out=hT[:], in_=h_ps[:], func=AF.Relu, bias=b1_sb[:, :1])
            v_ps = pC.tile([6, SEGC], F32, tag="sm")
            nc.tensor.matmul(v_ps[:], lhsT=w2_sb[:], rhs=hT[:], start=True, stop=True)

            dpart = ep.tile([P, 6], F32, tag="dpart")
            nc.vector.tensor_reduce(
                out=dpart[:],
                in_=de_sb[:].rearrange("p (a j) -> p j a", j=6),
                axis=AX.X,
                op=ALU.add,
            )
            m_ps = pC.tile([6, 1], F32, tag="sm2")
            nc.tensor.matmul(m_ps[:], lhsT=dpart[:], rhs=ones32[:], start=True, stop=True)
            meanv = ep.tile([6, 1], F32, tag="meanv")
            nc.scalar.mul(meanv[:], m_ps[:], 1.0 / 1024.0)
            b2m = ep.tile([6, 1], F32, tag="b2m")
            nc.vector.tensor_tensor(out=b2m[:], in0=b2_sb[:], in1=meanv[:], op=ALU.mult)
            ovt_sb = ep.tile([6, SEGC], F32, tag="ovt")
            nc.scalar.activation(
                out=ovt_sb[:],
                in_=v_ps[:],
                func=AF.Identity,
                bias=b2m[:, :1],
                scale=meanv[:, :1],
            )
            nc.sync.dma_start(ovt[:], ovt_sb[:])

            # norm head
            hn_ps = pC.tile([P, SEGC], F32, tag="work")
            nc.tensor.matmul(hn_ps[:], lhsT=w1n_sb[:], rhs=xsT[:], start=True, stop=True)
            hnT = ep.tile([P, SEGC], F32, tag="hT")
            nc.scalar.activation(
                out=hnT[:], in_=hn_ps[:], func=AF.Relu, bias=b1n_sb[:, :1]
            )
            n_ps = pC.tile([1, SEGC], F32, tag="sm")
            nc.tensor.matmul(n_ps[:], lhsT=w2n_sb[:], rhs=hnT[:], start=True, stop=True)

            dnpart = ep.tile([P, 1], F32, tag="dnpart")
            nc.vector.tensor_reduce(out=dnpart[:], in_=dn_sb[:], axis=AX.X, op=ALU.add)
            mn_ps = pC.tile([1, 1], F32, tag="sm2")
            nc.tensor.matmul(
                mn_ps[:], lhsT=dnpart[:], rhs=ones32[:], start=True, stop=True
            )
            meann = ep.tile([1, 1], F32, tag="meann")
            nc.scalar.mul(meann[:], mn_ps[:], 1.0 / 1024.0)
            b2nm = ep.tile([1, 1], F32, tag="b2nm")
            nc.vector.tensor_tensor(
                out=b2nm[:], in0=b2n_sb[:], in1=meann[:], op=ALU.mult
            )
            ont_sb = ep.tile([1, SEGC], F32, tag="ont")
            nc.scalar.activation(
                out=ont_sb[:],
                in_=n_ps[:],
                func=AF.Identity,
                bias=b2nm[:1, :1],
                scale=meann[:1, :1],
            )
            nc.sync.dma_start(ont[:], ont_sb[:])

    return nc


def prepare_inputs(x_clu, mask_clu, batch_clu, dist_embedding, dist_embedding_norm,
                   gate_w, gate_b, w1, b1, w2, b2, w1n, b1n, w2n, b2n):
    """Host-side sharding / layout. Returns (NB, in_maps)."""
    x = np.asarray(x_clu, dtype=np.float32)
    mask = np.asarray(mask_clu).astype(bool)
    seg = np.asarray(batch_clu).astype(np.int64)
    N = x.shape[0]

    bounds = np.searchsorted(seg, np.arange(0, B_SEG + 1, SEGC))
    cnts = np.diff(bounds)
    NB = int(np.ceil(cnts.max() / NPB))
    NB = ((NB + BPG - 1) // BPG) * BPG
    NG = NB // BPG
    NTOT = NB * NPB

    # shared (replicated) small tensors
    shared = {
        "wrep": np.ascontiguousarray(
            np.broadcast_to(
                np.asarray(gate_w, np.float32).reshape(-1)[None, :], (P, D)
            ).astype(np.float16)
        ),
        "w1h": np.ascontiguousarray(np.asarray(w1, np.float32)),
        "b1h": np.asarray(b1, np.float32).reshape(D, 1).copy(),
        "w2h": np.ascontiguousarray(np.asarray(w2, np.float32)),
        "b2h": np.asarray(b2, np.float32).reshape(6, 1).copy(),
        "w1nh": np.ascontiguousarray(np.asarray(w1n, np.float32)),
        "b1nh": np.asarray(b1n, np.float32).reshape(D, 1).copy(),
        "w2nh": np.ascontiguousarray(np.asarray(w2n, np.float32)),
        "b2nh": np.asarray(b2n, np.float32).reshape(1, 1).copy(),
        "deh": np.asarray(dist_embedding, np.float32).reshape(P, 48).copy(),
        "dnh": np.asarray(dist_embedding_norm, np.float32).reshape(P, 8).copy(),
    }

    in_maps = []
    for c in range(NCORES):
        lo, hi = int(bounds[c]), int(bounds[c + 1])
        cnt = hi - lo

        xp = np.zeros((NTOT, D + 1), np.float16)
        xp[:cnt, :D] = x[lo:hi].astype(np.float16)
        xp[:, D] = 1.0
        xh = np.ascontiguousarray(
            xp.reshape(NB, TPB, P, D + 1).transpose(0, 2, 1, 3)
        )

        rel = np.full(NTOT, -10_000, np.int64)
        rel[:cnt] = seg[lo:hi] - c * SEGC
        mk = np.zeros(NTOT, bool)
        mk[:cnt] = mask[lo:hi]

        relb = rel.reshape(NB, NPB)
        has_real = (relb >= 0).any(axis=1)
        LO = np.where(has_real, relb[:, 0], 0)
        # window span check (sorted ids -> max is at the last real node)
        span = np.where(has_real, relb.max(axis=1) - LO + 1, 1)
        assert span.max() <= WB, f"segment window {span.max()} exceeds WB={WB}"

        ohf = (
            (rel.reshape(NB, NPB, 1) - LO[:, None, None])
            == np.arange(WB)[None, None, :]
        ) & mk.reshape(NB, NPB, 1)
        ohh = np.ascontiguousarray(
            ohf.reshape(NB, TPB, P, WB).transpose(0, 2, 1, 3).astype(np.float16)
        )

        F = np.zeros((NG, P, SEGC), np.float16)
        for b in range(NB):
            if not has_real[b]:
                continue
            g, bb = divmod(b, BPG)
            js = np.arange(WB)
            ss = LO[b] + js
            valid = (ss >= 0) & (ss < SEGC)
            F[g, bb * GSTRIDE + js[valid], ss[valid]] = 1.0

        m = {"xh": xh, "ohh": ohh, "fh": F}
        m.update(shared)
        in_maps.append(m)

    return NB, bounds, in_maps


def _run(inputs: dict, trace: bool = False):
    NB, bounds, in_maps = prepare_inputs(**inputs)
    nc = build_program(NB)
    nc.finalize()
    res = run_bass_kernel_spmd(nc, in_maps, list(range(NCORES)), trace=trace)
    ov = np.concatenate([res.results[c]["ovt"].T for c in range(NCORES)], axis=0)
    on = np.concatenate([res.results[c]["ont"].T for c in range(NCORES)], axis=0)
    return (ov.astype(np.float32), on.astype(np.float32)), res


def kernel(**inputs):
    out, _ = _run(inputs, trace=False)
    return out
